# revision 2
# baseline (speedup 1.0000x reference)
import sys
if "/opt/trn_rl_repo" not in sys.path:
    sys.path.insert(0, "/opt/trn_rl_repo")
import hashlib
import numpy as np
import ml_dtypes
import jax
jax.config.update("jax_compilation_cache_dir", "/tmp/jax_kernel_cache")
jax.config.update("jax_persistent_cache_min_compile_time_secs", 0)
jax.config.update("jax_persistent_cache_min_entry_size_bytes", 0)
import concourse.bass as bass
from concourse import bacc
import concourse.tile as tile
from concourse import mybir

F32 = mybir.dt.float32
F32R = mybir.dt.float32r
BF16 = mybir.dt.bfloat16
AF = mybir.ActivationFunctionType
ALU = mybir.AluOpType

D = 512
H = 8
HD = 64
L = 2
IN = 16
S = 1024
BL = 2          # batch elems per core
NCORES = 8
LN_EPS = 1e-5
DELTA_SCALE = 1.5
NEG = -1.0e30

# fwT slots: 0=x (query_w.T), 1=fused q0, 2=fused k0, 3=fused k1
FX, FQ0, FK0, FK1 = range(4)
# wbig slots: 0-3 = wq1 (kt); W2 lives in w2t [16, L, H, D]
# smalls columns
XB, QB0F, KB0F, QB1, KB1F, OBP0, OBP1, LNG0, LNB0, LNG1, LNB1, OPW = \
    0, 4, 8, 12, 16, 20, 24, 28, 32, 36, 40, 44
OPB = 48
MSK = 56
IDC = 184
NSM = 200


def _build(consts, gates):
    nc = bacc.Bacc(None, target_bir_lowering=False, debug=False, num_devices=NCORES)
    featT_e = nc.declare_dram_parameter("featT", [BL, IN, S], BF16, isOutput=False)
    out_e = nc.declare_dram_parameter("out", [BL, S], F32, isOutput=True)
    fwT_e = nc.inline_tensor(consts["fw"], name="fwT")
    wbig_e = nc.inline_tensor(consts["wbig"], name="wbig")
    w2t_e = nc.inline_tensor(consts["w2t"], name="w2t")
    sm_e = nc.inline_tensor(consts["smalls"], name="smalls")
    with tile.TileContext(nc) as tc:
        _emit(nc, tc, gates, dict(featT=featT_e, fwT=fwT_e, wbig=wbig_e,
                                  w2t=w2t_e, sm=sm_e, out=out_e))
    nc.compile()
    jb = nc.to_json_bytes()
    nc.to_json_bytes = (lambda _jb=jb: _jb)
    return nc


def _emit(nc, tc, gates, E):
    from contextlib import ExitStack
    ctx = ExitStack()
    with ctx:
        P = bass.MemorySpace.PSUM
        wp = ctx.enter_context(tc.tile_pool(name="wp", bufs=1))
        feat_p = ctx.enter_context(tc.tile_pool(name="feat", bufs=1))
        fa_p = ctx.enter_context(tc.tile_pool(name="fa", bufs=1))
        x_p = ctx.enter_context(tc.tile_pool(name="x", bufs=1))
        xb_p = ctx.enter_context(tc.tile_pool(name="xb", bufs=1))
        y_p = ctx.enter_context(tc.tile_pool(name="y", bufs=1))
        q_p = ctx.enter_context(tc.tile_pool(name="q", bufs=1))
        k_p = ctx.enter_context(tc.tile_pool(name="k", bufs=1))
        gt_p = ctx.enter_context(tc.tile_pool(name="gt", bufs=1))
        pr_p = ctx.enter_context(tc.tile_pool(name="pr", bufs=1))
        x2_p = ctx.enter_context(tc.tile_pool(name="x2", bufs=1))
        tmp_p = ctx.enter_context(tc.tile_pool(name="tmp", bufs=1))
        bc_p = ctx.enter_context(tc.tile_pool(name="bc", bufs=1))
        sinv_p = ctx.enter_context(tc.tile_pool(name="sinv", bufs=1))
        row_p = ctx.enter_context(tc.tile_pool(name="row", bufs=1))
        psA = ctx.enter_context(tc.tile_pool(name="psA", bufs=2, space=P))
        psS = ctx.enter_context(tc.tile_pool(name="psS", bufs=2, space=P))
        psV = ctx.enter_context(tc.tile_pool(name="psV", bufs=2, space=P))
        psB = ctx.enter_context(tc.tile_pool(name="psB", bufs=2, space=P))

        # ---- persistent weights/consts ----
        fwT = wp.tile([IN, 4, D], BF16)
        wbig = wp.tile([128, 4, D], BF16)
        w2t = wp.tile([IN, L, H, D], BF16)
        sm = wp.tile([128, NSM], F32)
        ones64 = wp.tile([1, HD], F32)     # bcast lhsT across 64 parts
        ones128c0 = wp.tile([128, 1], F32)  # LN-sum lhsT
        ones128r0 = wp.tile([1, 128], F32)  # bcast lhsT across 128 parts

        g = nc.gpsimd
        g.dma_start(fwT[:], E["fwT"][:])
        g.dma_start(wbig[:], E["wbig"][:])
        g.dma_start(w2t[:], E["w2t"][:])
        g.dma_start(sm[:], E["sm"][:])
        g.memset(ones64[:], 1.0)
        g.memset(ones128c0[:], 1.0)
        g.memset(ones128r0[:], 1.0)
        ones16r = ones64[0:1, 0:16].bitcast(F32R)
        ones128c = ones128c0[:].bitcast(F32R)
        ones128r = ones128r0[:].bitcast(F32R)

        maskA = sm[:, MSK:MSK + 128]
        identb = wp.tile([IN, IN], BF16)
        nc.scalar.copy(identb[:], sm[0:IN, IDC:IDC + IN])
        ident = identb[:]

        for b in range(BL):
            featT = feat_p.tile([IN, S], BF16)
            g.dma_start(featT[:], E["featT"][b])

            # features in natural layout [t, c] + ones col at 32 (PSUM
            # partition-32 alignment for the softmax-sum row), zeros 16:32
            faN = fa_p.tile([128, 8, 33], BF16)
            for tt in range(8):
                ps = psA.tile([128, IN], F32, tag="a", padded_shape=[128, 512])
                nc.tensor.matmul(ps[:], featT[:, tt * 128:(tt + 1) * 128],
                                 ident, start=True, stop=True)
                nc.scalar.copy(faN[:, tt, 0:IN], ps[:])
            g.memset(faN[:, :, IN:32], 0.0)
            g.memset(faN[:, :, 32], 1.0)

            # residual stream x = features @ query_w.T + query_b
            xT = x_p.tile([128, 4, S], F32R)
            for dt in range(4):
                for qs in range(2):
                    cols = bass.ts(qs, 512)
                    ps = psA.tile([128, 512], F32, tag="a")
                    nc.tensor.matmul(ps[:], fwT[:, FX, dt * 128:(dt + 1) * 128],
                                     featT[:, cols], start=True, stop=True)
                    nc.scalar.activation(xT[:, dt, cols], ps[:], AF.Identity,
                                         bias=sm[:, XB + dt:XB + dt + 1])

            xb16 = None
            for l in range(L):
                # ---- q projection (transposed layout, bf16 out) ----
                qT = q_p.tile([128, 4, S], BF16)
                if l == 0:
                    for dt in range(4):
                        for qs in range(2):
                            cols = bass.ts(qs, 512)
                            ps = psA.tile([128, 512], F32, tag="a")
                            nc.tensor.matmul(
                                ps[:], fwT[:, FQ0, dt * 128:(dt + 1) * 128],
                                featT[:, cols], start=True, stop=True)
                            nc.scalar.activation(
                                qT[:, dt, cols], ps[:], AF.Identity,
                                bias=sm[:, QB0F + dt:QB0F + dt + 1])
                else:
                    for dt in range(4):
                        for qs in range(2):
                            cols = bass.ts(qs, 512)
                            ps = psA.tile([128, 512], F32, tag="a")
                            for kt in range(4):
                                nc.tensor.matmul(
                                    ps[:], wbig[:, kt, dt * 128:(dt + 1) * 128],
                                    xb16[:, kt, cols], start=(kt == 0),
                                    stop=(kt == 3))
                            nc.scalar.activation(
                                qT[:, dt, cols], ps[:], AF.Identity,
                                bias=sm[:, QB1 + dt:QB1 + dt + 1])

                # ---- k projection (fused rank-16, both layers) ----
                fk = FK0 if l == 0 else FK1
                kbc = KB0F if l == 0 else KB1F
                kT = k_p.tile([128, 4, S], BF16)
                for dt in range(4):
                    for qs in range(2):
                        cols = bass.ts(qs, 512)
                        ps = psA.tile([128, 512], F32, tag="a")
                        nc.tensor.matmul(ps[:], fwT[:, fk, dt * 128:(dt + 1) * 128],
                                         featT[:, cols], start=True, stop=True)
                        nc.scalar.activation(kT[:, dt, cols], ps[:], AF.Identity,
                                             bias=sm[:, kbc + dt:kbc + dt + 1])

                # ---- attention: G_h = softmax(qk) @ [F|1] per head ----
                gT = gt_p.tile([IN, H, S], BF16)
                for h in range(H):
                    hp = (h % 2) * 64
                    dht = h // 2
                    for qblk in range(2):
                        probsT = pr_p.tile([128, 8, 512], BF16)
                        pv = psV.tile([33, 512], F32, tag="v")
                        nkj = 4 * (qblk + 1)
                        for kj in range(nkj):
                            off = max(0, (kj - 4 * qblk) * 128)
                            sc = psS.tile([128, 512], F32, tag="s")
                            nc.tensor.matmul(
                                sc[:, off:],
                                kT[hp:hp + 64, dht, kj * 128:(kj + 1) * 128],
                                qT[hp:hp + 64, dht, qblk * 512 + off:(qblk + 1) * 512],
                                start=True, stop=True)
                            if kj >= 4 * qblk:
                                nc.vector.tensor_add(sc[:, off:off + 128],
                                                     sc[:, off:off + 128], maskA)
                            nc.scalar.activation(probsT[:, kj, off:], sc[:, off:],
                                                 AF.Exp, scale=0.125)
                            nc.tensor.matmul(pv[:, off:], faN[:, kj, :],
                                             probsT[:, kj, off:],
                                             start=(kj == 0), stop=(kj == nkj - 1))
                        # normalize by softmax sums (row 32 of pv)
                        srow = row_p.tile([1, 512], F32R, bufs=2)
                        nc.scalar.copy(srow[:], pv[32:33, :])
                        sb = psB.tile([IN, 512], F32, tag="b")
                        nc.tensor.matmul(sb[:], ones16r, srow[:],
                                         start=True, stop=True)
                        sinv = sinv_p.tile([IN, 512], F32)
                        nc.vector.reciprocal(sinv[:], sb[:])
                        cols = bass.ts(qblk, 512)
                        nc.vector.tensor_mul(gT[:, h, cols],
                                             pv[0:IN, :], sinv[:])

                # ---- attn out = sum_h G_h @ W2_h (+obp) + residual add ----
                yT = y_p.tile([128, 4, S], F32R)
                obc = OBP0 if l == 0 else OBP1
                for dt in range(4):
                    for qs in range(2):
                        cols = bass.ts(qs, 512)
                        ps = psA.tile([128, 512], F32, tag="a")
                        for h in range(H):
                            nc.tensor.matmul(
                                ps[:], w2t[:, l, h, dt * 128:(dt + 1) * 128],
                                gT[:, h, cols], start=(h == 0), stop=(h == H - 1))
                        nc.vector.scalar_tensor_tensor(
                            yT[:, dt, cols], ps[:], sm[:, obc + dt:obc + dt + 1],
                            xT[:, dt, cols].bitcast(F32),
                            op0=ALU.add, op1=ALU.add)

                # ---- layernorm ----
                lngc = LNG0 if l == 0 else LNG1
                lnbc = LNB0 if l == 0 else LNB1
                xT = x_p.tile([128, 4, S], F32R)
                if l == 0:
                    xb16 = xb_p.tile([128, 4, S], BF16)
                for qs in range(2):
                    cols = bass.ts(qs, 512)
                    mps = psS.tile([1, 512], F32, tag="s", padded_shape=None)
                    for dt in range(4):
                        nc.tensor.matmul(mps[:], ones128c, yT[:, dt, cols],
                                         start=(dt == 0), stop=(dt == 3))
                    vps = psS.tile([1, 512], F32, tag="s", padded_shape=None)
                    for dt in range(4):
                        x2 = x2_p.tile([128, 512], F32R)
                        nc.scalar.activation(x2[:], yT[:, dt, cols].bitcast(F32),
                                             AF.Square)
                        nc.tensor.matmul(vps[:], ones128c, x2[:],
                                         start=(dt == 0), stop=(dt == 3))
                    mrow = row_p.tile([1, 512], F32R)
                    nc.scalar.mul(mrow[:], mps[:], 1.0 / D)
                    s1 = row_p.tile([1, 512], F32)
                    nc.vector.tensor_mul(s1[:], mrow[:].bitcast(F32),
                                         mrow[:].bitcast(F32))
                    s2 = row_p.tile([1, 512], F32)
                    nc.vector.scalar_tensor_tensor(
                        s2[:], vps[:], 1.0 / D, s1[:],
                        op0=ALU.mult, op1=ALU.subtract)
                    s4 = row_p.tile([1, 512], F32)
                    nc.vector.tensor_scalar_add(s4[:], s2[:], LN_EPS)
                    s3 = row_p.tile([1, 512], F32)
                    nc.scalar.sqrt(s3[:], s4[:])
                    rrowr = row_p.tile([1, 512], F32R)
                    with nc.allow_low_precision(reason="f32r rstd"):
                        nc.vector.reciprocal(rrowr[:], s3[:])
                    # broadcast mean and rstd to 128 partitions
                    mbps = psB.tile([128, 512], F32, tag="b")
                    nc.tensor.matmul(mbps[:], ones128r, mrow[:],
                                     start=True, stop=True)
                    mbc = bc_p.tile([128, 512], F32)
                    nc.scalar.copy(mbc[:], mbps[:])
                    rbps = psB.tile([128, 512], F32, tag="b")
                    nc.tensor.matmul(rbps[:], ones128r, rrowr[:],
                                     start=True, stop=True)
                    rbc = bc_p.tile([128, 512], F32)
                    nc.scalar.copy(rbc[:], rbps[:])
                    for dt in range(4):
                        tmp = tmp_p.tile([128, 512], F32)
                        nc.vector.tensor_sub(tmp[:], yT[:, dt, cols].bitcast(F32),
                                             mbc[:])
                        nc.vector.scalar_tensor_tensor(
                            xT[:, dt, cols], tmp[:], sm[:, lngc + dt:lngc + dt + 1],
                            rbc[:], op0=ALU.mult, op1=ALU.mult)
                        if gates["lnb"]:
                            nc.vector.tensor_scalar_add(
                                xT[:, dt, cols], xT[:, dt, cols],
                                sm[:, lnbc + dt:lnbc + dt + 1])
                        if l == 0:
                            nc.scalar.copy(xb16[:, dt, cols], xT[:, dt, cols])

            # ---- final projection + tanh ----
            for qs in range(2):
                cols = bass.ts(qs, 512)
                fps = psS.tile([1, 512], F32, tag="s", padded_shape=None)
                for dt in range(4):
                    nc.tensor.matmul(fps[:],
                                     sm[:, OPW + dt:OPW + dt + 1].bitcast(F32R),
                                     xT[:, dt, cols], start=(dt == 0),
                                     stop=(dt == 3))
                th = row_p.tile([1, 512], F32)
                nc.scalar.activation(th[:], fps[:], AF.Tanh,
                                     bias=sm[0:1, OPB:OPB + 1])
                orow = row_p.tile([1, 512], F32)
                nc.scalar.mul(orow[:], th[:], DELTA_SCALE)
                nc.sync.dma_start(E["out"][b:b + 1, cols], orow[:])


def _host_pack(inputs):
    f32, f64 = np.float32, np.float64
    ip = {k: np.asarray(v, f32) for k, v in inputs.items()}
    featT = np.ascontiguousarray(
        ip["features"].transpose(0, 2, 1).astype(ml_dtypes.bfloat16))  # [B, IN, S]

    hw, hb = ip["hist_w"].astype(f64), ip["hist_b"].astype(f64)
    qw, qb = ip["query_w"].astype(f64), ip["query_b"].astype(f64)
    ipw, ipb = ip["in_proj_w"].astype(f64), ip["in_proj_b"].astype(f64)
    wo, ob = ip["attn_out_w"].astype(f64), ip["attn_out_b"].astype(f64)
    wq, wk, wv = ipw[:, :D], ipw[:, D:2 * D], ipw[:, 2 * D:]
    bq, bk, bv = ipb[:, :D], ipb[:, D:2 * D], ipb[:, 2 * D:]

    fw = np.stack([
        qw.T,               # x residual stream
        (wq[0] @ qw).T,     # fused q layer 0
        (wk[0] @ hw).T,     # fused k layer 0
        (wk[1] @ hw).T,     # fused k layer 1
    ], axis=1).astype(ml_dtypes.bfloat16)  # [16, 4, 512]

    def packw1(w):  # [512(out),512(in)] -> [128,4,512] lhsT layout
        return np.ascontiguousarray(w.T.reshape(4, 128, D).transpose(1, 0, 2))

    # W2_l[h*16+c, d] = (wo_l[:, h-dims] @ (wv_l @ hw)[h-dims, :])[d, c]
    def w2(l):
        fvw = wv[l] @ hw  # [512, 16]
        out = np.zeros((128, D), f64)
        for h in range(H):
            out[h * IN:(h + 1) * IN] = (wo[l][:, h * HD:(h + 1) * HD]
                                        @ fvw[h * HD:(h + 1) * HD, :]).T
        return out

    wbig = packw1(wq[1]).astype(ml_dtypes.bfloat16)  # [128, 4, 512]
    # w2t[c, l, h, d] = W2_l[h*16+c, d]
    w2t = np.stack([w2(0), w2(1)], axis=0)  # [L, 128, D]
    w2t = np.ascontiguousarray(
        w2t.reshape(L, H, IN, D).transpose(2, 0, 1, 3)).astype(
        ml_dtypes.bfloat16)  # [16, L, H, D]

    def colpack(v):  # [512] -> [128, 4]
        return np.asarray(v, f64).reshape(4, 128).T

    vb0f = hb @ wv[0].T + bv[0]
    vb1f = hb @ wv[1].T + bv[1]
    sm = np.zeros((128, NSM), f64)
    sm[:, XB:XB + 4] = colpack(qb)
    sm[:, QB0F:QB0F + 4] = colpack(qb @ wq[0].T + bq[0])
    sm[:, KB0F:KB0F + 4] = colpack(hb @ wk[0].T + bk[0])
    sm[:, QB1:QB1 + 4] = colpack(bq[1])
    sm[:, KB1F:KB1F + 4] = colpack(hb @ wk[1].T + bk[1])
    sm[:, OBP0:OBP0 + 4] = colpack(vb0f @ wo[0].T + ob[0])
    sm[:, OBP1:OBP1 + 4] = colpack(vb1f @ wo[1].T + ob[1])
    sm[:, LNG0:LNG0 + 4] = colpack(ip["ln_g"][0])
    sm[:, LNB0:LNB0 + 4] = colpack(ip["ln_b"][0])
    sm[:, LNG1:LNG1 + 4] = colpack(ip["ln_g"][1])
    sm[:, LNB1:LNB1 + 4] = colpack(ip["ln_b"][1])
    sm[:, OPW:OPW + 4] = colpack(ip["out_proj_w"][0])
    sm[0, OPB] = ip["out_proj_b"][0]
    rows = np.arange(128)[:, None]
    colsi = np.arange(128)[None, :]
    sm[:, MSK:MSK + 128] = np.where(rows > colsi, NEG, 0.0)
    sm[0:IN, IDC:IDC + IN] = np.eye(IN)
    sm = sm.astype(f32)

    consts = dict(fw=fw, wbig=np.ascontiguousarray(wbig),
                  w2t=np.ascontiguousarray(w2t),
                  smalls=np.ascontiguousarray(sm))
    gates = dict(lnb=bool(np.any(ip["ln_b"] != 0.0)))
    return featT, consts, gates


_CACHE = {}


def build_and_inmaps(inputs):
    featT, consts, gates = _host_pack(inputs)
    hsh = hashlib.sha1()
    for k in sorted(consts):
        hsh.update(consts[k].tobytes())
    key = (hsh.hexdigest(), gates["lnb"])
    if key not in _CACHE:
        _CACHE[key] = _build(consts, gates)
    nc = _CACHE[key]
    in_maps = [
        {"featT": np.ascontiguousarray(featT[c * BL:(c + 1) * BL])}
        for c in range(NCORES)
    ]
    return nc, in_maps


def kernel(**inputs):
    from concourse.bass_utils import run_bass_kernel_spmd
    nc, in_maps = build_and_inmaps(inputs)
    res = run_bass_kernel_spmd(nc, in_maps, list(range(NCORES)))
    outs = [res.results[c]["out"] for c in range(NCORES)]
    return np.concatenate(outs, axis=0).astype(np.float32)
